# revision 1
# baseline (speedup 1.0000x reference)
"""Masked dot-product attention on 8 TRN2 NeuronCores (Bass/Tile).

Strategy (decided from the fixed problem shape B=16, NQ=NK=2048, D=DV=128):

* The softmax key-mask (valid_lens) makes the work per batch proportional to
  ceil(L_b/128) k-tiles.  Total work is split into per-core "slots": every
  core runs the same SPMD program of S slots with fixed tile-extents
  [e_0..e_S-1]; the host assigns each (core, slot) one contiguous
  (batch, k-range) segment at kernel-call time (recomputed from the actual
  valid_lens, so any input works).  Segments of one batch may land on
  different cores/slots; the host sums the partial results.

* Device math per k-tile t (128 keys), per q-half (1024 queries):
    S^T[k,q]   = K_tile @ Q^T          (float32r matmuls, full PE rate)
    P^T[k,q]   = exp(S^T/sqrt(D) + bias_k)   (one ScalarE op: scale+bias+exp
                 fused; bias is -1e6 for masked/padded keys so P underflows
                 to exactly 0 -> no max-subtraction, no separate masking)
    O^T[v,q]  += V_tile^T-contraction matmul (bf16)     [PSUM accumulate]
    d[1,q]    += ones^T @ P^T                (bf16)     [PSUM accumulate]
  Working entirely in the transposed (k-on-partitions) orientation means no
  on-device transposes at all: the host pre-transposes Q and K once.

* The host divides the accumulated numerator by the denominator and
  transposes back.
"""

import math

import ml_dtypes
import numpy as np

import concourse.bass as bass  # noqa: F401  (bass types used via tile/bacc)
import concourse.mybir as mybir
import concourse.tile as tile
from concourse import bacc
from concourse.bass_utils import run_bass_kernel_spmd

B, NQ, NK, D, DV = 16, 2048, 2048, 128, 128
NCORES = 8
KT = 128  # keys per k-tile (partition dim)
QH = 1024  # queries per q-half (PSUM sizing)
NEG = np.float32(-1.0e6)
SCALE = 1.0 / math.sqrt(D)

F32 = mybir.dt.float32
F32R = mybir.dt.float32r
BF16 = mybir.dt.bfloat16

# QK matmul precision: float32r (TF32-like, ~5e-4 score err) or bf16
# (~6e-3 score err, faster LDWEIGHTS + half the Q/K DMA bytes)
QK_BF16 = False

_PROGRAM_CACHE: dict[tuple, object] = {}
LAST_RESULT = None  # BassKernelResults of the most recent run (for test.py)


# ---------------------------------------------------------------- scheduling
def _try_fill(sizes, extents, n_cores):
    """Greedy: at each slot position give every core a segment of the batch
    with the most remaining tiles.  Returns assign[core][pos] = (batch,
    tile_start, n_tiles) or None, or None if infeasible."""
    rem = list(sizes)
    nxt = [0] * len(sizes)
    assign = [[None] * len(extents) for _ in range(n_cores)]
    for p, e in enumerate(extents):
        for c in range(n_cores):
            b = int(np.argmax(rem))
            if rem[b] <= 0:
                continue
            seg = min(rem[b], e)
            assign[c][p] = (b, nxt[b], seg)
            nxt[b] += seg
            rem[b] -= seg
    if any(r > 0 for r in rem):
        return None
    return assign


def _schedule(sizes, n_cores=NCORES):
    """Pick slot extents minimizing executed tiles per core."""
    total = sum(sizes)
    cap = (total + n_cores - 1) // n_cores
    best = None
    # enumerate descending extent tuples with small total slack
    def gen(prefix, remaining, maxpart, nleft):
        if nleft == 0:
            if remaining == 0:
                yield tuple(prefix)
            return
        lo = (remaining + nleft - 1) // nleft
        for e in range(min(maxpart, remaining - nleft + 1), lo - 1, -1):
            yield from gen(prefix + [e], remaining - e, e, nleft - 1)

    # measured A/B: an extra slot costs ~8us (DMA restart + evictions) vs
    # ~2.7us per pad tile -> strongly prefer fewer slots, then fewer tiles
    for nslots in range(1, 7):
        for slack in range(0, 4):
            tot = cap + slack
            cands = sorted(
                gen([], tot, min(tot, max(sizes)), nslots),
                key=lambda t: -min(t),  # prefer balanced extents
            )
            for extents in cands:
                a = _try_fill(sizes, extents, n_cores)
                if a is not None:
                    best = (tot, extents, a)
                    break
            if best is not None:
                break
        if best is not None:
            break
    if best is None:  # fallback: any feasible
        for slack in range(4, cap + 1):
            tot = cap + slack
            for nslots in range(1, 7):
                for extents in gen([], tot, min(tot, max(sizes)), nslots):
                    a = _try_fill(sizes, extents, n_cores)
                    if a is not None:
                        best = (tot, extents, a)
                        break
                if best is not None:
                    break
            if best is not None:
                break
    assert best is not None, "scheduler failed"
    return list(best[1]), best[2]


# ------------------------------------------------------------ device program
def _build(extents):
    nc = bacc.Bacc()
    S = len(extents)
    QKDT = BF16 if QK_BF16 else F32R
    qt_d, kt_d, v_d, b_d, o_d, d_d = [], [], [], [], [], []
    for s, e in enumerate(extents):
        qt_d.append(nc.dram_tensor(f"qt{s}", [D, NQ], QKDT, kind="ExternalInput"))
        # K^T tiled [tile][d][k-within-tile] so each tile is one contiguous DMA
        kt_d.append(nc.dram_tensor(f"kt{s}", [e, D, KT], QKDT, kind="ExternalInput"))
        # V pre-blocked on host to SBUF image [k-within-tile, tile*DV]
        v_d.append(nc.dram_tensor(f"v{s}", [KT, e * DV], BF16, kind="ExternalInput"))
        b_d.append(nc.dram_tensor(f"b{s}", [KT, e], F32, kind="ExternalInput"))
        o_d.append(nc.dram_tensor(f"o{s}", [DV, NQ], F32, kind="ExternalOutput"))
        d_d.append(nc.dram_tensor(f"d{s}", [1, NQ], F32, kind="ExternalOutput"))

    with tile.TileContext(nc) as tc:
        with (
            tc.tile_pool(name="const", bufs=1) as cpool,
            tc.tile_pool(name="qt", bufs=2) as qpool,
            tc.tile_pool(name="kt", bufs=2) as kpool,
            tc.tile_pool(name="v", bufs=2) as vpool,
            tc.tile_pool(name="bias", bufs=2) as bpool,
            tc.tile_pool(name="p", bufs=8) as ppool,
            tc.tile_pool(name="ps", bufs=3) as pspool,
            tc.tile_pool(name="osb", bufs=2) as opool_sb,
            tc.tile_pool(name="dsb", bufs=2) as dpool_sb,
            tc.tile_pool(name="spsum", bufs=2, space="PSUM") as spool,
            tc.tile_pool(name="opsum", bufs=1, space="PSUM") as opool,
            tc.tile_pool(name="dpsum", bufs=1, space="PSUM") as dpool,
        ):
            ones = cpool.tile([KT, 1], BF16)
            nc.vector.memset(ones[:], 1.0)
            # warmup: trigger exp ACT-table load + PE HAM ramp during input DMA
            wsrc = cpool.tile([KT, 128], BF16)
            nc.vector.memset(wsrc[:], 0.0)
            wpt = ppool.tile([KT, QH], BF16, tag="pt")
            nc.scalar.activation(
                wpt[:, :128], wsrc[:], mybir.ActivationFunctionType.Exp
            )
            wps = dpool.tile([1, QH], F32, tag="dpsum")
            for _ in range(8):
                nc.tensor.matmul(wps[:, :128], ones[:], wsrc[:], start=True, stop=True)

            for s, e in enumerate(extents):
                # ordering matters: tiny bias first, then the regions the
                # first tiles need; v/bias ride a second DMA queue
                bias = bpool.tile([KT, 16], F32, tag="bias")
                nc.gpsimd.dma_start(bias[:, :e], b_d[s][:])
                qt = qpool.tile([D, NQ], QKDT, tag="qt")
                kt = kpool.tile([D, 16 * KT], QKDT, tag="kt")
                vt = vpool.tile([KT, 16 * KT], BF16, tag="v")
                nc.gpsimd.dma_start(vt[:, : e * DV], v_d[s][:])
                nc.sync.dma_start(qt[:, 0:512], qt_d[s][:, 0:512])
                nc.sync.dma_start(kt[:, :KT], kt_d[s][0])
                nc.sync.dma_start(qt[:, 512:1024], qt_d[s][:, 512:1024])
                for t in range(1, e):
                    nc.sync.dma_start(kt[:, t * KT : (t + 1) * KT], kt_d[s][t])
                for c4 in (2, 3):
                    nc.sync.dma_start(
                        qt[:, c4 * 512 : (c4 + 1) * 512],
                        qt_d[s][:, c4 * 512 : (c4 + 1) * 512],
                    )

                for h in range(2):
                    q0 = h * QH
                    opsum = opool.tile([DV, QH], F32)
                    dpsum = dpool.tile([1, QH], F32)
                    pts = [None] * e
                    # software-pipelined: S-matmuls run one tile ahead of
                    # the PV/denominator matmuls that consume exp's output
                    for i in range(e + 1):
                        if i < e:
                            t = i
                            spsum = spool.tile([KT, QH], F32)
                            for c in range(2):
                                nc.tensor.matmul(
                                    spsum[:, c * 512 : (c + 1) * 512],
                                    kt[:, t * KT : (t + 1) * KT],
                                    qt[:, q0 + c * 512 : q0 + (c + 1) * 512],
                                    start=True,
                                    stop=True,
                                )
                            pt = ppool.tile([KT, QH], BF16)
                            nc.scalar.activation(
                                pt[:],
                                spsum[:],
                                mybir.ActivationFunctionType.Exp,
                                bias=bias[:, t : t + 1],
                                scale=SCALE,
                            )
                            pts[t] = pt
                        if i > 0:
                            t = i - 1
                            pt = pts[t]
                            first, last = t == 0, t == e - 1
                            for c in range(2):
                                nc.tensor.matmul(
                                    opsum[:, c * 512 : (c + 1) * 512],
                                    vt[:, t * KT : (t + 1) * KT],
                                    pt[:, c * 512 : (c + 1) * 512],
                                    start=first,
                                    stop=last,
                                )
                            # denominator: sum tile-pairs on DVE (bf16 2x),
                            # then one ones-matmul per pair into dpsum
                            if t % 2 == 1:
                                ps = pspool.tile([KT, QH], BF16, tag="ps")
                                nc.vector.tensor_tensor(
                                    ps[:], pts[t - 1][:], pt[:], mybir.AluOpType.add
                                )
                                for c in range(2):
                                    nc.tensor.matmul(
                                        dpsum[:, c * 512 : (c + 1) * 512],
                                        ones[:],
                                        ps[:, c * 512 : (c + 1) * 512],
                                        start=(t == 1),
                                        stop=(t == e - 1),
                                    )
                            elif t == e - 1:  # odd tail (or e == 1)
                                for c in range(2):
                                    nc.tensor.matmul(
                                        dpsum[:, c * 512 : (c + 1) * 512],
                                        ones[:],
                                        pt[:, c * 512 : (c + 1) * 512],
                                        start=(e == 1),
                                        stop=True,
                                    )
                    osb = opool_sb.tile([DV, QH], F32, tag="osb")
                    for c in range(2):  # chunked: DMA chunk 0 under copy 1
                        nc.vector.tensor_copy(
                            osb[:, c * 512 : (c + 1) * 512],
                            opsum[:, c * 512 : (c + 1) * 512],
                        )
                        nc.sync.dma_start(
                            o_d[s][:, q0 + c * 512 : q0 + (c + 1) * 512],
                            osb[:, c * 512 : (c + 1) * 512],
                        )
                    dsb = dpool_sb.tile([1, QH], F32, tag="dsb")
                    nc.vector.tensor_copy(dsb[:], dpsum[:])
                    nc.sync.dma_start(d_d[s][:, q0 : q0 + QH], dsb[:])
    nc.compile()
    return nc


# ------------------------------------------------------------------- kernel
def kernel(queries, keys, values, valid_lens, _trace=False):
    global LAST_RESULT
    queries = np.asarray(queries, dtype=np.float32)
    keys = np.asarray(keys, dtype=np.float32)
    values = np.asarray(values, dtype=np.float32)
    valid_lens = np.asarray(valid_lens, dtype=np.int32)

    sizes = [int((int(l) + KT - 1) // KT) for l in valid_lens]
    extents, assign = _schedule(sizes)
    key = tuple(extents)
    if key not in _PROGRAM_CACHE:
        _PROGRAM_CACHE[key] = _build(extents)
    nc = _PROGRAM_CACHE[key]

    qk_np = ml_dtypes.bfloat16 if QK_BF16 else np.float32
    qT = np.ascontiguousarray(queries.transpose(0, 2, 1)).astype(qk_np)  # [B,D,NQ]
    kT = np.ascontiguousarray(keys.transpose(0, 2, 1)).astype(qk_np)  # [B,D,NK]
    v_bf = values.astype(ml_dtypes.bfloat16)  # [B, NK, DV]
    # bias column per (batch, tile-row): 0 where key position valid else -1e6
    pos = np.arange(NK, dtype=np.int32).reshape(NK // KT, KT)  # [tiles, 128]
    bias_all = np.where(
        pos[None] < valid_lens[:, None, None], np.float32(0.0), NEG
    ).astype(np.float32)  # [B, tiles, 128]

    in_maps = []
    for c in range(NCORES):
        m = {}
        for s, e in enumerate(extents):
            seg = assign[c][s]
            qt = np.zeros((D, NQ), qk_np)
            kt = np.zeros((e, D, KT), qk_np)
            vv = np.zeros((KT, e * DV), ml_dtypes.bfloat16)
            bb = np.full((KT, e), NEG, np.float32)
            if seg is not None:
                b, t0, n = seg
                qt[:] = qT[b]
                kt[:n] = (
                    kT[b][:, t0 * KT : (t0 + n) * KT]
                    .reshape(D, n, KT)
                    .transpose(1, 0, 2)
                )
                # [n*KT, DV] -> SBUF image [KT, n*DV] (k-within-tile major)
                vv[:, : n * DV] = (
                    v_bf[b][t0 * KT : (t0 + n) * KT]
                    .reshape(n, KT, DV)
                    .transpose(1, 0, 2)
                    .reshape(KT, n * DV)
                )
                bb[:, :n] = bias_all[b][t0 : t0 + n].T
            m[f"qt{s}"] = qt
            m[f"kt{s}"] = kt
            m[f"v{s}"] = vv
            m[f"b{s}"] = bb
        in_maps.append(m)

    res = run_bass_kernel_spmd(
        nc, in_maps, core_ids=list(range(NCORES)), trace=_trace
    )
    LAST_RESULT = res

    o_acc = np.zeros((B, DV, NQ), np.float32)
    d_acc = np.zeros((B, NQ), np.float32)
    for c in range(NCORES):
        for s in range(len(extents)):
            seg = assign[c][s]
            if seg is None:
                continue
            b = seg[0]
            o_acc[b] += res.results[c][f"o{s}"]
            d_acc[b] += res.results[c][f"d{s}"][0]

    out = (o_acc / d_acc[:, None, :]).transpose(0, 2, 1)
    return np.ascontiguousarray(out.astype(np.float32))

